# revision 1
# baseline (speedup 1.0000x reference)
"""Trainium2 Bass kernel for 2-layer GATv2 + output projection (SPMD, 8 cores).

Strategy: nodes partitioned across cores; edges sorted by destination and
packed into fixed 128-node windows (J=5 tiles of 512 edge slots each, window
addressing static so the SPMD program is uniform). Per tile: edge features +
xr[dst] accumulate in PSUM via matmuls (selector matrices built on-device),
gathered xl[src] (dma_gather, two int16-half calls) transposed into the same
PSUM; leaky-relu; per-head logits via PE; exp on ACT; segment softmax
denominators + weighted scatter-add via selector matmuls accumulated per
window. Halo exchange = AllGather of xl shards. fp32r (TF32-like) matmuls
with host/DVE-rounded operands; fp32 accumulate.
"""
import numpy as np

import concourse.bass as bass
import concourse.bacc as bacc
import concourse.mybir as mybir
import concourse.tile as tile
from concourse.bass_utils import run_bass_kernel_spmd
from concourse.masks import make_identity

F32 = mybir.dt.float32
F32R = mybir.dt.float32r
I16 = mybir.dt.int16
I32 = mybir.dt.int32

NEG_SLOPE = 0.2
EPS = 1e-30


def f32r_round(x):
    b = np.ascontiguousarray(x, np.float32).view(np.uint32)
    q = (b + 0x7FF + ((b >> 12) & 1)) & np.uint32(0xFFFFF000)
    return q.view(np.float32)


class Cfg:
    def __init__(self, N, E, IN_F, NC, J):
        self.N, self.E, self.IN_F, self.NC, self.J = N, E, IN_F, NC, J
        self.F = 256
        self.H, self.C = 4, 64
        self.W = 128                      # nodes per window
        assert N % NC == 0
        self.NV = N // NC                 # nodes per core
        self.NW = (self.NV + self.W - 1) // self.W
        self.NVP = self.NW * self.W       # padded nodes per core
        self.T_E = 512                    # edge slots per tile
        self.G = 4                        # 128-edge groups per tile
        self.SW = self.J * self.T_E       # edge slots per window
        self.NT = self.NW * self.J        # tiles per core
        self.NFULL = self.NC * self.NVP   # rows in gathered xl table
        assert self.NFULL % 2 == 0
        self.HALF = self.NFULL // 2
        assert self.HALF <= 32768, "int16 gather index range"
        self.KCH = self.IN_F // 128       # K-chunks for layer-0 projection
        import os
        _ph = os.environ.get("K_PHASES", "p1,ag0,e0,p4,ag1,e1")
        self.phases = tuple(x for x in _ph.split(",") if x)


def preprocess(cfg, edge_index, edge_attr):
    """Sort edges by dst, pack into windows/tiles, build per-core arrays."""
    src = np.asarray(edge_index[0], np.int64)
    dst = np.asarray(edge_index[1], np.int64)
    ea = np.asarray(edge_attr, np.float32)
    NV, NVP, W, NW, J, SW, T_E, NT = (cfg.NV, cfg.NVP, cfg.W, cfg.NW, cfg.J,
                                      cfg.SW, cfg.T_E, cfg.NT)
    # padded-global row of each node's xl entry
    core_of = src // NV
    grow = core_of * NVP + (src - core_of * NV)

    order = np.argsort(dst, kind="stable")
    cores = []
    for c in range(cfg.NC):
        lo = np.searchsorted(dst, c * NV, side="left", sorter=order)
        hi = np.searchsorted(dst, (c + 1) * NV, side="left", sorter=order)
        eidx_c = order[lo:hi]
        dloc = dst[eidx_c] - c * NV
        win = dloc // W

        TC = T_E - 1                      # real-edge capacity per tile
        idx_hi = np.zeros((NT, T_E), np.int16)
        idx_lo = np.full((NT, T_E), -1, np.int16)
        klow = np.zeros(NT, np.int32)
        dstf = np.full((NT, T_E), -1.0, np.float32)
        eat = np.zeros((NT, T_E, ea.shape[1]), np.float32)

        for w in range(NW):
            e_w = eidx_c[win == w]
            g_w = grow[e_w]
            o = np.argsort(g_w, kind="stable")
            e_w, g_w = e_w[o], g_w[o]
            n_e = len(e_w)
            assert n_e <= J * TC, f"window overflow: {n_e} > {J * TC}; raise J"
            for j in range(J):
                t = w * J + j
                ec = e_w[j * TC:(j + 1) * TC]
                gc = g_w[j * TC:(j + 1) * TC]
                ne = len(ec)
                kl = int((gc < cfg.HALF).sum()) + 1   # + leading dummy slot
                idx_lo[t, 0] = 0
                idx_lo[t, 1:kl] = gc[:kl - 1].astype(np.int16)
                idx_hi[t, kl:kl + (ne - kl + 1)] = (gc[kl - 1:] - cfg.HALF
                                                    ).astype(np.int16)
                klow[t] = kl
                if ne:
                    dstf[t, 1:ne + 1] = (dst[ec] - c * NV - w * W
                                         ).astype(np.float32)
                    eat[t, 1:ne + 1] = ea[ec]

        wrap = lambda a: np.tile(a.reshape(NT, T_E // 16, 16).transpose(0, 2, 1),
                                 (1, 8, 1)).copy()
        # per-tile layouts
        dstf_t = dstf.reshape(NT, 4, 128).transpose(0, 2, 1).copy()
        dstrow = dstf.copy()                        # [NT, 512]
        eat_t = f32r_round(np.ascontiguousarray(eat.transpose(0, 2, 1)))
        cores.append(dict(idx_hi=wrap(idx_hi), idx_lo=wrap(idx_lo),
                          klow=klow.reshape(1, NT), dstf=dstf_t,
                          dstrow=dstrow, eat=eat_t))
    return cores


def build_program(cfg, nc):
    """Emit the full SPMD program into nc (a Bacc) under TileContext."""
    F, G, T_E, J, NW, NT, NVP, W = (cfg.F, cfg.G, cfg.T_E, cfg.J, cfg.NW,
                                    cfg.NT, cfg.NVP, cfg.W)
    EF = 32
    # ---- external inputs
    P = {}
    def inp(name, shape, dt):
        P[name] = nc.dram_tensor(name, shape, dt, kind="ExternalInput")
        return P[name]

    x_T = inp("x_T", [cfg.IN_F, NVP], F32R)
    idx_hi = inp("idx_hi", [NT, 128, T_E // 16], I16)
    idx_lo = inp("idx_lo", [NT, 128, T_E // 16], I16)
    klow = inp("klow", [1, NT], I32)
    dstf = inp("dstf", [NT, 128, G], F32)
    dstrow = inp("dstrow", [NT, T_E], F32)
    eat = inp("eat", [NT, EF, T_E], F32R)
    wl0 = inp("wl0", [cfg.IN_F, F], F32R)
    wr0 = inp("wr0", [cfg.IN_F, F], F32R)
    we0 = inp("we0", [EF, F], F32R)
    wl1 = inp("wl1", [F, F], F32R)
    wr1 = inp("wr1", [F, F], F32R)
    we1 = inp("we1", [EF, F], F32R)
    wout = inp("wout", [F, 1], F32)
    att0 = inp("att0", [128, 2, 4], F32)
    att1 = inp("att1", [128, 2, 4], F32)
    bl0 = inp("bl0", [128, F], F32)
    br0 = inp("br0", [128, F], F32)
    bias0 = inp("bias0", [128, F], F32)
    bl1 = inp("bl1", [128, F], F32)
    br1 = inp("br1", [128, F], F32)
    bias1 = inp("bias1", [128, F], F32)
    bout = inp("bout", [128, 1], F32)
    iota_r = inp("iota_r", [128, 128], F32)
    iota_c = inp("iota_c", [128, 1], F32)

    out_own = nc.dram_tensor("out_own", [NVP, 1], F32, kind="ExternalOutput")

    # ---- internal DRAM
    xl0_own = nc.dram_tensor("xl0_own", [NVP, F], F32R)
    xr0_own = nc.dram_tensor("xr0_own", [NVP, F], F32R)
    xl1_own = nc.dram_tensor("xl1_own", [NVP, F], F32R)
    xr1_own = nc.dram_tensor("xr1_own", [NVP, F], F32R)
    akw = dict(addr_space="Shared") if cfg.NC > 4 else {}
    xl0_full = nc.dram_tensor("xl0_full", [cfg.NFULL, F], F32R, **akw)
    xl1_full = nc.dram_tensor("xl1_full", [cfg.NFULL, F], F32R, **akw)
    h1_own = nc.dram_tensor("h1_own", [NVP, F], F32)
    groups = [list(range(cfg.NC))]

    with tile.TileContext(nc) as tc:
        with (
            tc.tile_pool(name="const", bufs=1) as constp,
            tc.tile_pool(name="wpool", bufs=1) as wpool,
            tc.tile_pool(name="io", bufs=4) as io,
            tc.tile_pool(name="sel", bufs=2 * J) as selp,
            tc.tile_pool(name="mpool", bufs=2) as mpool,
            tc.tile_pool(name="small", bufs=4) as small,
            tc.tile_pool(name="psA", bufs=2, space="PSUM") as psA,
            tc.tile_pool(name="psW", bufs=2, space="PSUM") as psW,
            tc.tile_pool(name="psS", bufs=2, space="PSUM") as psS,
        ):
            ident = constp.tile([128, 128], F32)
            make_identity(nc, ident[:])
            iota_row = constp.tile([128, 128], F32)
            nc.sync.dma_start(out=iota_row[:], in_=iota_r[:])
            iota_col = constp.tile([128, 1], F32)
            nc.sync.dma_start(out=iota_col[:], in_=iota_c[:])
            klow_sb = constp.tile([1, NT], I32)
            nc.sync.dma_start(out=klow_sb[:], in_=klow[:])
            batt = {}
            for nm, t in (("att0", att0), ("att1", att1), ("bout", bout)):
                sh = [128, 2, 4] if nm.startswith("att") else [128, 1]
                bt = constp.tile(sh, F32, tag=nm)
                nc.sync.dma_start(out=bt[:], in_=t[:])
                batt[nm] = bt
            bsb = {}
            for nm, t in (("bl0", bl0), ("br0", br0), ("bias0", bias0),
                          ("bl1", bl1), ("br1", br1), ("bias1", bias1)):
                bt = constp.tile([128, F], F32, tag=nm)
                nc.sync.dma_start(out=bt[:], in_=t[:])
                bsb[nm] = bt

            def load_w(t, kdim, tag):
                n = kdim // 128
                w = wpool.tile([128, n, F], F32R, tag=tag)
                for k in range(n):
                    nc.sync.dma_start(out=w[:, k, :], in_=t[k * 128:(k + 1) * 128, :])
                return w
            wl0_sb = load_w(wl0, cfg.IN_F, "wl0")
            wr0_sb = load_w(wr0, cfg.IN_F, "wr0")
            wl1_sb = load_w(wl1, F, "wl1")
            wr1_sb = load_w(wr1, F, "wr1")
            we0_sb = wpool.tile([EF, F], F32R, tag="we0")
            nc.sync.dma_start(out=we0_sb[:], in_=we0[:])
            we1_sb = wpool.tile([EF, F], F32R, tag="we1")
            nc.sync.dma_start(out=we1_sb[:], in_=we1[:])
            wout_sb = wpool.tile([128, 2, 1], F32, tag="wout")
            for h in range(2):
                nc.sync.dma_start(out=wout_sb[:, h, :], in_=wout[h * 128:(h + 1) * 128, :])

            # ---------------- P1: layer-0 projections ----------------
            for c in range(NW if "p1" in cfg.phases else 0):
                xk = io.tile([128, cfg.KCH, 128], F32R, tag="xk")
                for k in range(cfg.KCH):
                    nc.sync.dma_start(
                        out=xk[:, k, :],
                        in_=x_T[k * 128:(k + 1) * 128, c * 128:(c + 1) * 128])
                pl = psA.tile([128, F], F32, tag="pm")
                pr = psA.tile([128, F], F32, tag="pm")
                for k in range(cfg.KCH):
                    nc.tensor.matmul(pl[:], lhsT=xk[:, k, :], rhs=wl0_sb[:, k, :],
                                     start=(k == 0), stop=(k == cfg.KCH - 1),
                                     skip_group_check=True)
                for k in range(cfg.KCH):
                    nc.tensor.matmul(pr[:], lhsT=xk[:, k, :], rhs=wr0_sb[:, k, :],
                                     start=(k == 0), stop=(k == cfg.KCH - 1),
                                     skip_group_check=True)
                ol = io.tile([128, F], F32R, tag="oxl")
                orr = io.tile([128, F], F32R, tag="oxr")
                nc.vector.tensor_add(out=ol[:], in0=pl[:], in1=bsb["bl0"][:])
                nc.vector.tensor_add(out=orr[:], in0=pr[:], in1=bsb["br0"][:])
                nc.sync.dma_start(out=xl0_own[c * 128:(c + 1) * 128, :], in_=ol[:])
                nc.sync.dma_start(out=xr0_own[c * 128:(c + 1) * 128, :], in_=orr[:])

            if "ag0" in cfg.phases:
                tc.strict_bb_all_engine_barrier()
                nc.gpsimd.collective_compute(
                    "AllGather", mybir.AluOpType.bypass, replica_groups=groups,
                    ins=[xl0_own[:]], outs=[xl0_full[:]])
                tc.strict_bb_all_engine_barrier()

            # ---------------- edge pass (shared for both layers) ------------
            def edge_pass(layer, xl_full, xr_own, we_sb, att_sb, bias_sb):
                for w in range(NW):
                    xr_win = io.tile([128, F], F32R, tag="xrw")
                    nc.sync.dma_start(out=xr_win[:],
                                      in_=xr_own[w * W:(w + 1) * W, :])
                    pd = psW.tile([128, 4], F32, tag="wacc")
                    s_ts, st_ts, ex_es, stages = [], [], [], []
                    for j in range(J):
                        t = w * J + j
                        reg = nc.gpsimd.alloc_register()
                        nc.gpsimd.load(reg, klow_sb[0:1, t:t + 1])
                        stage = selp.tile([128, G, F], F32R, tag="stage")
                        iht = io.tile([128, T_E // 16], I16, tag="ih")
                        ilt = io.tile([128, T_E // 16], I16, tag="il")
                        nc.sync.dma_start(out=iht[:], in_=idx_hi[t])
                        nc.sync.dma_start(out=ilt[:], in_=idx_lo[t])
                        nc.gpsimd.dma_gather(
                            out_ap=stage[:], in_ap=xl_full[cfg.HALF:, :],
                            idxs_ap=iht[:], num_idxs=T_E, num_idxs_reg=T_E,
                            elem_size=F)
                        nc.gpsimd.dma_gather(
                            out_ap=stage[:], in_ap=xl_full[:cfg.HALF, :],
                            idxs_ap=ilt[:], num_idxs=T_E, num_idxs_reg=reg,
                            elem_size=F)
                        ea_t = io.tile([EF, T_E], F32R, tag="ea")
                        nc.sync.dma_start(out=ea_t[:], in_=eat[t])
                        dstf_t = io.tile([128, G], F32, tag="dstf")
                        nc.sync.dma_start(out=dstf_t[:], in_=dstf[t])
                        drow = io.tile([128, T_E], F32, tag="drow")
                        nc.sync.dma_start(
                            out=drow[:],
                            in_=dstrow[t:t + 1, :].to_broadcast([128, T_E]))
                        s_t = selp.tile([128, G, 128], F32R, tag="s")
                        for g in range(G):
                            nc.vector.tensor_tensor(
                                out=s_t[:, g, :],
                                in0=dstf_t[:, g:g + 1].to_broadcast([128, 128]),
                                in1=iota_row[:], op=mybir.AluOpType.is_equal)
                        st_t = selp.tile([128, T_E], F32R, tag="st")
                        nc.vector.tensor_tensor(
                            out=st_t[:], in0=iota_col[:].to_broadcast([128, T_E]),
                            in1=drow[:], op=mybir.AluOpType.is_equal)
                        pm = psA.tile([128, 2, T_E], F32, tag="pm")
                        for h in range(2):
                            nc.tensor.matmul(
                                pm[:, h, :], lhsT=we_sb[:, h * 128:(h + 1) * 128],
                                rhs=ea_t[:], start=True, stop=False,
                                skip_group_check=True)
                            nc.tensor.matmul(
                                pm[:, h, :],
                                lhsT=xr_win[:, h * 128:(h + 1) * 128],
                                rhs=st_t[:], start=False, stop=False,
                                skip_group_check=True)
                        for g in range(G):
                            for h in range(2):
                                nc.tensor.matmul(
                                    pm[:, h, g * 128:(g + 1) * 128],
                                    lhsT=stage[:, g, h * 128:(h + 1) * 128].bitcast(F32),
                                    rhs=ident[:], is_transpose=True,
                                    start=False, stop=(g == G - 1),
                                    skip_group_check=True)
                        m_t = mpool.tile([128, 2, T_E], F32, tag="m")
                        rp = mpool.tile([128, 2, T_E], F32, tag="rp")
                        for h in range(2):
                            nc.scalar.activation(
                                rp[:, h, :], pm[:, h, :],
                                mybir.ActivationFunctionType.Relu,
                                scale=1.0 - NEG_SLOPE)
                            nc.vector.scalar_tensor_tensor(
                                out=m_t[:, h, :], in0=pm[:, h, :],
                                scalar=NEG_SLOPE, in1=rp[:, h, :],
                                op0=mybir.AluOpType.mult,
                                op1=mybir.AluOpType.add)
                        plog = psS.tile([128, 16], F32, tag="sm")
                        for g in range(G):
                            for h in range(2):
                                nc.tensor.matmul(
                                    plog[:, g * 4:(g + 1) * 4],
                                    lhsT=m_t[:, h, g * 128:(g + 1) * 128],
                                    rhs=att_sb[:, h, :],
                                    start=(h == 0), stop=(h == 1),
                                    skip_group_check=True)
                        ex_e = selp.tile([128, 16], F32, tag="ex")
                        nc.scalar.activation(ex_e[:], plog[:],
                                             mybir.ActivationFunctionType.Exp)
                        for g in range(G):
                            nc.tensor.matmul(
                                pd[:], lhsT=s_t[:, g, :].bitcast(F32),
                                rhs=ex_e[:, g * 4:(g + 1) * 4],
                                start=(j == 0 and g == 0),
                                stop=(j == J - 1 and g == G - 1),
                                skip_group_check=True)
                        s_ts.append(s_t); st_ts.append(st_t); ex_es.append(ex_e); stages.append(stage)
                    rdf = small.tile([128, 4], F32, tag="rdf")
                    nc.vector.tensor_scalar_add(out=rdf[:], in0=pd[:], scalar1=EPS)
                    rden = small.tile([128, 4], F32, tag="rden")
                    nc.vector.reciprocal(out=rden[:], in_=rdf[:])
                    pagg = psW.tile([128, F], F32, tag="wacc")
                    for j in range(J):
                        s_t, st_t, ex_e = s_ts[j], st_ts[j], ex_es[j]
                        stage = stages[j]
                        pr = psS.tile([128, 16], F32, tag="sm")
                        for g in range(G):
                            nc.tensor.matmul(
                                pr[:, g * 4:(g + 1) * 4],
                                lhsT=st_t[:, g * 128:(g + 1) * 128].bitcast(F32),
                                rhs=rden[:], start=True, stop=True,
                                skip_group_check=True)
                        alpha = small.tile([128, 16], F32, tag="alpha")
                        nc.vector.tensor_tensor(out=alpha[:], in0=ex_e[:],
                                                in1=pr[:],
                                                op=mybir.AluOpType.mult)
                        for g in range(G):
                            v = small.tile([128, 4, 64], F32R, tag="v")
                            nc.vector.tensor_tensor(
                                out=v[:],
                                in0=stage[:, g, :].bitcast(F32)
                                    .rearrange("p (h c) -> p h c", h=4),
                                in1=alpha[:, g * 4:(g + 1) * 4].unsqueeze(-1)
                                    .to_broadcast([128, 4, 64]),
                                op=mybir.AluOpType.mult)
                            nc.tensor.matmul(
                                pagg[:], lhsT=s_t[:, g, :],
                                rhs=v[:].rearrange("p h c -> p (h c)"),
                                start=(j == 0 and g == 0),
                                stop=(j == J - 1 and g == G - 1),
                                skip_group_check=True)
                    hsum = small.tile([128, F], F32, tag="hsum")
                    nc.vector.tensor_add(out=hsum[:], in0=pagg[:], in1=bias_sb[:])
                    h_out = small.tile([128, F], F32, tag="hout")
                    nc.scalar.activation(h_out[:], hsum[:],
                                         mybir.ActivationFunctionType.Relu)
                    if layer == 0:
                        nc.sync.dma_start(out=h1_own[w * W:(w + 1) * W, :],
                                          in_=h_out[:])
                    else:
                        po = psW.tile([128, 1], F32, tag="wacc")
                        for h in range(2):
                            pt = psS.tile([128, 128], F32, tag="sm")
                            nc.tensor.matmul(pt[:], lhsT=h_out[:, h * 128:(h + 1) * 128],
                                             rhs=ident[:], is_transpose=True,
                                             start=True, stop=True,
                                             skip_group_check=True)
                            h2T = small.tile([128, 128], F32, tag="h2T")
                            nc.vector.tensor_copy(out=h2T[:], in_=pt[:])
                            nc.tensor.matmul(po[:], lhsT=h2T[:],
                                             rhs=wout_sb[:, h, :],
                                             start=(h == 0), stop=(h == 1),
                                             skip_group_check=True)
                        o_sb = small.tile([128, 1], F32, tag="osb")
                        nc.vector.tensor_scalar(
                            out=o_sb[:], in0=po[:], scalar1=batt["bout"][:, :1],
                            scalar2=None, op0=mybir.AluOpType.add)
                        nc.sync.dma_start(out=out_own[w * W:(w + 1) * W, :],
                                          in_=o_sb[:])

            if "e0" in cfg.phases:
                edge_pass(0, xl0_full, xr0_own, we0_sb, batt["att0"], bsb["bias0"])
            tc.strict_bb_all_engine_barrier()

            # ---------------- P4: layer-1 projections ----------------
            for c in range(NW if "p4" in cfg.phases else 0):
                h1c = io.tile([128, F], F32, tag="h1c")
                nc.sync.dma_start(out=h1c[:], in_=h1_own[c * 128:(c + 1) * 128, :])
                h1T = io.tile([128, 2, 128], F32R, tag="h1T")
                for h in range(2):
                    pt = psS.tile([128, 128], F32, tag="sm")
                    nc.tensor.matmul(pt[:], lhsT=h1c[:, h * 128:(h + 1) * 128],
                                     rhs=ident[:], is_transpose=True,
                                     start=True, stop=True, skip_group_check=True)
                    nc.vector.tensor_copy(out=h1T[:, h, :], in_=pt[:])
                pl = psA.tile([128, F], F32, tag="pm")
                pr = psA.tile([128, F], F32, tag="pm")
                for h in range(2):
                    nc.tensor.matmul(pl[:], lhsT=h1T[:, h, :], rhs=wl1_sb[:, h, :],
                                     start=(h == 0), stop=(h == 1),
                                     skip_group_check=True)
                for h in range(2):
                    nc.tensor.matmul(pr[:], lhsT=h1T[:, h, :], rhs=wr1_sb[:, h, :],
                                     start=(h == 0), stop=(h == 1),
                                     skip_group_check=True)
                ol = io.tile([128, F], F32R, tag="oxl")
                orr = io.tile([128, F], F32R, tag="oxr")
                nc.vector.tensor_add(out=ol[:], in0=pl[:], in1=bsb["bl1"][:])
                nc.vector.tensor_add(out=orr[:], in0=pr[:], in1=bsb["br1"][:])
                nc.sync.dma_start(out=xl1_own[c * 128:(c + 1) * 128, :], in_=ol[:])
                nc.sync.dma_start(out=xr1_own[c * 128:(c + 1) * 128, :], in_=orr[:])

            if "ag1" in cfg.phases:
                tc.strict_bb_all_engine_barrier()
                nc.gpsimd.collective_compute(
                    "AllGather", mybir.AluOpType.bypass, replica_groups=groups,
                    ins=[xl1_own[:]], outs=[xl1_full[:]])
                tc.strict_bb_all_engine_barrier()
            if "e1" in cfg.phases:
                edge_pass(1, xl1_full, xr1_own, we1_sb, batt["att1"], bsb["bias1"])
    return P


_CACHE = {}


def _get_compiled(cfg):
    key = (cfg.N, cfg.E, cfg.IN_F, cfg.NC, cfg.J)
    if key not in _CACHE:
        nc = bacc.Bacc("TRN2", target_bir_lowering=False, debug=False,
                       num_devices=cfg.NC)
        build_program(cfg, nc)
        nc.compile()
        _CACHE[key] = nc
    return _CACHE[key]


def make_in_maps(cfg, inputs, cores_pre):
    """Per-core input dicts."""
    x = np.asarray(inputs["x"], np.float32)
    H, C, F = cfg.H, cfg.C, cfg.F
    att_blk = {}
    for li in (0, 1):
        att = np.asarray(inputs[f"att{li}"], np.float32)   # [H, C]
        A = np.zeros((2 * 128, 4), np.float32)
        for h in range(H):
            A[h * C:(h + 1) * C, h] = att[h]
        att_blk[li] = np.ascontiguousarray(A.reshape(2, 128, 4).transpose(1, 0, 2))
    iota_r = np.tile(np.arange(128, dtype=np.float32)[None, :], (128, 1))
    iota_c = np.arange(128, dtype=np.float32).reshape(128, 1)
    rep = lambda v: np.tile(np.asarray(v, np.float32)[None, :], (128, 1))
    common = dict(
        wl0=f32r_round(inputs["W_l0"]), wr0=f32r_round(inputs["W_r0"]),
        we0=f32r_round(inputs["W_e0"]), wl1=f32r_round(inputs["W_l1"]),
        wr1=f32r_round(inputs["W_r1"]), we1=f32r_round(inputs["W_e1"]),
        wout=np.asarray(inputs["W_out"], np.float32),
        att0=att_blk[0], att1=att_blk[1],
        bl0=rep(inputs["b_l0"]), br0=rep(inputs["b_r0"]),
        bias0=rep(inputs["bias0"]), bl1=rep(inputs["b_l1"]),
        br1=rep(inputs["b_r1"]), bias1=rep(inputs["bias1"]),
        bout=np.tile(np.asarray(inputs["b_out"], np.float32).reshape(1, 1),
                     (128, 1)),
        iota_r=iota_r, iota_c=iota_c,
    )
    in_maps = []
    for c in range(cfg.NC):
        pre = cores_pre[c]
        xs = np.zeros((cfg.NVP, cfg.IN_F), np.float32)
        xs[:cfg.NV] = x[c * cfg.NV:(c + 1) * cfg.NV]
        m = dict(common)
        m.update(x_T=f32r_round(np.ascontiguousarray(xs.T)),
                 idx_hi=pre["idx_hi"], idx_lo=pre["idx_lo"],
                 klow=pre["klow"], dstf=pre["dstf"], dstrow=pre["dstrow"],
                 eat=pre["eat"])
        in_maps.append(m)
    return in_maps


def kernel(**inputs):
    cfg = Cfg(N=50000, E=800000, IN_F=512, NC=8, J=5)
    # bump J if some window overflows (keeps NEFF cache stable otherwise)
    dst = np.asarray(inputs["edge_index"][1], np.int64)
    loc = dst % cfg.NV
    wid = (dst // cfg.NV) * cfg.NW + loc // cfg.W
    need = np.bincount(wid, minlength=cfg.NC * cfg.NW).max()
    while need > cfg.J * (cfg.T_E - 1):
        cfg = Cfg(N=50000, E=800000, IN_F=512, NC=8, J=cfg.J + 1)
    cores_pre = preprocess(cfg, inputs["edge_index"], inputs["edge_attr"])
    in_maps = make_in_maps(cfg, inputs, cores_pre)
    nc = _get_compiled(cfg)
    res = run_bass_kernel_spmd(nc, in_maps, core_ids=list(range(cfg.NC)))
    outs = []
    for c in range(cfg.NC):
        outs.append(res.results[c]["out_own"][:cfg.NV])
    return np.concatenate(outs, 0).astype(np.float32)



# revision 31
# speedup vs baseline: 13.1432x; 13.1432x over previous
"""Trainium2 Bass kernel for 2-layer GATv2 + output projection (SPMD, 8 cores).

v2 strategy: nodes partitioned across cores; per 128-dst-node window, edges
are packed into SW=J*512 slots (sorted by gathered-row id so the low/high
int16 gather split is two window-level dma_gather calls). fp16 data plane
(same mantissa as TF32/f32r), f32 PSUM accumulation. Softmax denominator is
factored out of the weighted scatter (out = (sum ex*xl[src]) / denom), so
each window is a single pass: per 512-slot tile, edge-feature projection +
xr[dst] broadcast (selector matmul) + transposed gathered xl accumulate into
PSUM; fused leaky-relu; per-head logits via PE; exp on ACT; one selector
matmul per 128-slot group scatters both the weighted values and the
denominator (values carry ex in 4 extra columns). Halo exchange = AllGather
of fp16 xl shards. xr and the inter-layer h1 stay SBUF-resident.
"""
import numpy as np

import concourse.bass as bass
import concourse.bacc as bacc
import concourse.mybir as mybir
import concourse.tile as tile
from concourse.bass_utils import run_bass_kernel_spmd
from concourse.masks import make_identity

F32 = mybir.dt.float32
F16 = mybir.dt.float16
I16 = mybir.dt.int16
I32 = mybir.dt.int32

NEG_SLOPE = 0.2
EPS = 1e-30


class Cfg:
    def __init__(self, N, E, IN_F, NC, J, reps=1):
        self.N, self.E, self.IN_F, self.NC, self.J = N, E, IN_F, NC, J
        self.reps = reps
        self.F = 256
        self.H, self.C = 4, 64
        self.W = 128                      # dst nodes per window
        assert N % NC == 0
        self.NV = N // NC                 # nodes per core
        self.NW = (self.NV + self.W - 1) // self.W
        self.NVP = self.NW * self.W       # padded nodes per core
        self.T_E = 512                    # edge slots per tile
        self.G = 4                        # 128-edge groups per tile
        self.SW = self.J * self.T_E       # edge slots per window
        self.SWC = self.SW // 128         # 128-slot chunks per window
        import os
        self.NCH = int(os.environ.get("K_GCH", "4"))  # gather chunks/window
        assert self.SWC % self.NCH == 0
        self.HSW = self.SW // self.NCH    # slots per gather chunk
        assert self.HSW % 128 == 0
        self.NFULL = self.NC * self.NVP   # rows in gathered xl table
        assert self.NFULL % 2 == 0
        self.HALF = self.NFULL // 2
        assert self.HALF <= 32768, "int16 gather index range"
        self.KCH = self.IN_F // 128       # K-chunks for layer-0 projection
        import os
        _ph = os.environ.get("K_PHASES", "p1,ag0,e0,p4,ag1,e1")
        self.phases = tuple(x for x in _ph.split(",") if x)


def preprocess(cfg, edge_index, edge_attr):
    """Sort edges by dst window, pack into window slots, build per-core arrays."""
    src = np.asarray(edge_index[0], np.int64)
    dst = np.asarray(edge_index[1], np.int64)
    ea = np.asarray(edge_attr, np.float32)
    NV, W, NW, SW, SWC = cfg.NV, cfg.W, cfg.NW, cfg.SW, cfg.SWC
    EF = ea.shape[1]
    # padded-global row of each node's xl entry
    core_of = src // NV
    grow = core_of * cfg.NVP + (src - core_of * NV)

    order = np.argsort(dst, kind="stable")
    wrap = lambda a: np.tile(a.reshape(len(a) // 16, 16).T, (8, 1))
    cores = []
    for c in range(cfg.NC):
        lo = np.searchsorted(dst, c * NV, side="left", sorter=order)
        hi = np.searchsorted(dst, (c + 1) * NV, side="left", sorter=order)
        eidx_c = order[lo:hi]
        dloc = dst[eidx_c] - c * NV
        win = dloc // W

        HSW, NCH = cfg.HSW, cfg.NCH
        idx2 = np.zeros((NW, 128, 2, NCH, HSW // 16), np.int16)
        klow = np.zeros((1, NCH * NW), np.int32)
        dstf = np.zeros((NW, 128, SWC), np.float16)
        eat = np.zeros((NW, EF + 1, SW), np.float16)

        for w in range(NW):
            e_w = eidx_c[win == w]
            g_w = grow[e_w]
            o = np.argsort(g_w, kind="stable")
            e_w, g_w = e_w[o], g_w[o]
            n_e = len(e_w)
            cap = HSW - 1
            assert n_e <= NCH * cap, (
                f"window overflow: {n_e} > {NCH * cap}; raise J")
            drow = np.full(SW, -1.0, np.float16)
            eaT = np.zeros((EF, SW), np.float16)
            dl = (dst[e_w] - c * NV - w * W).astype(np.float16)
            for q in range(NCH):
                e_q = e_w[q * cap:(q + 1) * cap]
                g_q = g_w[q * cap:(q + 1) * cap]
                d_q = dl[q * cap:(q + 1) * cap]
                n_q = len(e_q)
                n_lo = int((g_q < cfg.HALF).sum())
                kl = n_lo + 1             # + leading dummy slot
                idx_lo = np.full(HSW, -1, np.int16)
                idx_lo[0] = 0
                idx_lo[1:kl] = g_q[:n_lo].astype(np.int16)
                idx_hi = np.zeros(HSW, np.int16)
                idx_hi[kl:kl + (n_q - n_lo)] = (g_q[n_lo:] - cfg.HALF
                                                ).astype(np.int16)
                idx2[w, :, 0, q, :] = wrap(idx_hi)
                idx2[w, :, 1, q, :] = wrap(idx_lo)
                klow[0, NCH * w + q] = kl
                s0 = q * HSW
                drow[s0 + 1:s0 + kl] = d_q[:n_lo]
                drow[s0 + kl:s0 + kl + (n_q - n_lo)] = d_q[n_lo:]
                eaT[:, s0 + 1:s0 + kl] = ea[e_q[:n_lo]].T
                eaT[:, s0 + kl:s0 + kl + (n_q - n_lo)] = ea[e_q[n_lo:]].T
            dstf[w] = drow.reshape(SWC, 128).T
            eat[w, EF, :] = drow
            eat[w, :EF, :] = eaT
        cores.append(dict(idx2=idx2, klow=klow, dstf=dstf, eat=eat))
    return cores


def build_program(cfg, nc):
    """Emit the full SPMD program into nc (a Bacc) under TileContext."""
    F, G, T_E, J, NW, SW, SWC, NVP, W = (cfg.F, cfg.G, cfg.T_E, cfg.J, cfg.NW,
                                         cfg.SW, cfg.SWC, cfg.NVP, cfg.W)
    EF = 32
    P = {}
    def inp(name, shape, dt):
        P[name] = nc.dram_tensor(name, shape, dt, kind="ExternalInput")
        return P[name]

    x_T = inp("x_T", [cfg.IN_F, NVP], F16)
    idx2 = inp("idx2", [NW, 128, 2, cfg.NCH, cfg.HSW // 16], I16)
    klow = inp("klow", [1, cfg.NCH * NW], I32)
    dstf = inp("dstf", [NW, 128, SWC], F16)
    eat = inp("eat", [NW, EF + 1, SW], F16)
    wl0 = inp("wl0", [cfg.IN_F, F], F16)
    wr0 = inp("wr0", [cfg.IN_F, F], F16)
    we0 = inp("we0", [EF, F], F16)
    wl1 = inp("wl1", [F, F], F16)
    wr1 = inp("wr1", [F, F], F16)
    we1 = inp("we1", [EF, F], F16)
    wout = inp("wout", [128, 2, 1], F16)
    att0 = inp("att0", [128, 2, 4], F16)
    att1 = inp("att1", [128, 2, 4], F16)
    bl0 = inp("bl0", [128, F], F32)
    br0 = inp("br0", [128, F], F32)
    bias0 = inp("bias0", [128, F], F32)
    bl1 = inp("bl1", [128, F], F32)
    br1 = inp("br1", [128, F], F32)
    bias1 = inp("bias1", [128, F], F32)
    bout = inp("bout", [128, 1], F32)
    iota_r16 = inp("iota_r16", [128, 128], F16)
    iota_c = inp("iota_c", [128, 1], F32)
    ones16 = inp("ones16", [33, 128], F16)
    ident_in = inp("ident_in", [128, 128], F16)

    out_own = nc.dram_tensor("out_own", [NVP, 1], F32, kind="ExternalOutput")

    # ---- internal DRAM
    xl0_own = nc.dram_tensor("xl0_own", [NVP, F], F16)
    xl1_own = nc.dram_tensor("xl1_own", [NVP, F], F16)
    akw = dict(addr_space="Shared") if cfg.NC > 4 else {}
    xl0_full = nc.dram_tensor("xl0_full", [cfg.NFULL, F], F16, **akw)
    xl1_full = nc.dram_tensor("xl1_full", [cfg.NFULL, F], F16, **akw)
    groups = [list(range(cfg.NC))]

    with tile.TileContext(nc) as tc:
        with (
            tc.tile_pool(name="const", bufs=1) as constp,
            tc.tile_pool(name="wpool", bufs=1) as wpool,
            tc.tile_pool(name="resid", bufs=1) as resid,
            tc.tile_pool(name="io", bufs=3) as io,
            tc.tile_pool(name="stg", bufs=2) as stg,
            tc.tile_pool(name="sel", bufs=3) as selp,
            tc.tile_pool(name="mpool", bufs=2) as mpool,
            tc.tile_pool(name="small", bufs=4) as small,
            tc.tile_pool(name="psA", bufs=2, space="PSUM") as psA,
            tc.tile_pool(name="psW", bufs=2, space="PSUM") as psW,
            tc.tile_pool(name="psS", bufs=2, space="PSUM") as psS,
        ):
            ident16 = constp.tile([128, 128], F16)
            nc.sync.dma_start(out=ident16[:], in_=ident_in[:])
            iota_row = constp.tile([128, 128], F16)
            nc.sync.dma_start(out=iota_row[:], in_=iota_r16[:])
            iota_col = constp.tile([128, 1], F32)
            nc.sync.dma_start(out=iota_col[:], in_=iota_c[:])
            ones_sb = constp.tile([33, 128], F16)
            nc.sync.dma_start(out=ones_sb[:], in_=ones16[:])
            klow_sb = constp.tile([1, cfg.NCH * NW], I32)
            nc.sync.dma_start(out=klow_sb[:], in_=klow[:])
            batt = {}
            for nm, t in (("att0", att0), ("att1", att1)):
                bt = constp.tile([128, 2, 4], F16, tag=nm)
                nc.sync.dma_start(out=bt[:], in_=t[:])
                batt[nm] = bt
            bout_sb = constp.tile([128, 1], F32)
            nc.sync.dma_start(out=bout_sb[:], in_=bout[:])
            bsb = {}
            for nm, t in (("bl0", bl0), ("br0", br0), ("bias0", bias0),
                          ("bl1", bl1), ("br1", br1), ("bias1", bias1)):
                bt = constp.tile([128, F], F32, tag=nm)
                nc.sync.dma_start(out=bt[:], in_=t[:])
                bsb[nm] = bt

            def load_w(t, kdim, tag):
                n = kdim // 128
                w = wpool.tile([128, n, F], F16, tag=tag)
                nc.sync.dma_start(
                    out=w[:], in_=t.rearrange("(k p) f -> p k f", p=128))
                return w
            wl0_sb = load_w(wl0, cfg.IN_F, "wl0")
            wr0_sb = load_w(wr0, cfg.IN_F, "wr0")
            wl1_sb = load_w(wl1, F, "wl1")
            wr1_sb = load_w(wr1, F, "wr1")
            we0_sb = wpool.tile([EF, F], F16, tag="we0")
            nc.sync.dma_start(out=we0_sb[:], in_=we0[:])
            we1_sb = wpool.tile([EF, F], F16, tag="we1")
            nc.sync.dma_start(out=we1_sb[:], in_=we1[:])
            wout_sb = wpool.tile([128, 2, 1], F16, tag="wout")
            nc.sync.dma_start(out=wout_sb[:], in_=wout[:])

            # SBUF-resident xr (shared by both layers) and inter-layer h1
            xr_sb = resid.tile([128, NW, F], F16, tag="xr")
            h1_sb = resid.tile([128, NW, F], F16, tag="h1")

            # ---------------- layer-0 projections ----------------
            def phase_p1():
                for c in range(NW):
                    xk = io.tile([128, cfg.KCH, 128], F16, tag="xk")
                    nc.sync.dma_start(
                        out=xk[:],
                        in_=x_T.rearrange("(k p) n -> p k n", p=128)
                            [:, :, c * 128:(c + 1) * 128])
                    plr = psA.tile([128, 2, F], F32, tag="pm")
                    for k in range(cfg.KCH):
                        nc.tensor.matmul(plr[:, 0, :], lhsT=xk[:, k, :],
                                         rhs=wl0_sb[:, k, :],
                                         start=(k == 0), stop=(k == cfg.KCH - 1),
                                         skip_group_check=True)
                    for k in range(cfg.KCH):
                        nc.tensor.matmul(plr[:, 1, :], lhsT=xk[:, k, :],
                                         rhs=wr0_sb[:, k, :],
                                         start=(k == 0), stop=(k == cfg.KCH - 1),
                                         skip_group_check=True)
                    ol = io.tile([128, F], F16, tag="oxl")
                    nc.vector.tensor_add(out=ol[:], in0=plr[:, 0, :],
                                         in1=bsb["bl0"][:])
                    nc.vector.tensor_add(out=xr_sb[:, c, :], in0=plr[:, 1, :],
                                         in1=bsb["br0"][:])
                    nc.sync.dma_start(out=xl0_own[c * 128:(c + 1) * 128, :],
                                      in_=ol[:])

            # ---------------- edge pass (shared for both layers) ------------
            def edge_pass(layer, xl_full, we_sb, att_sb, bias_sb):
                NCH = cfg.NCH
                HC = SWC // NCH
                for w in range(NW):
                    idx_sb = io.tile([128, 2, NCH, cfg.HSW // 16], I16,
                                     tag="idx")
                    nc.sync.dma_start(out=idx_sb[:], in_=idx2[w])
                    ea_sb = io.tile([EF + 1, SW], F16, tag="ea")
                    nc.sync.dma_start(out=ea_sb[:], in_=eat[w])
                    dstf_sb = io.tile([128, SWC], F16, tag="dstf")
                    nc.sync.dma_start(out=dstf_sb[:], in_=dstf[w])
                    stage = stg.tile([128, SWC, F], F16, tag="stage")
                    for q in range(NCH):
                        reg = nc.gpsimd.alloc_register()
                        nc.gpsimd.load(
                            reg, klow_sb[0:1, NCH * w + q:NCH * w + q + 1])
                        nc.gpsimd.dma_gather(
                            out_ap=stage[:, q * HC:(q + 1) * HC, :],
                            in_ap=xl_full[cfg.HALF:, :],
                            idxs_ap=idx_sb[:, 0, q, :], num_idxs=cfg.HSW,
                            num_idxs_reg=cfg.HSW, elem_size=F)
                        nc.gpsimd.dma_gather(
                            out_ap=stage[:, q * HC:(q + 1) * HC, :],
                            in_ap=xl_full[:cfg.HALF, :],
                            idxs_ap=idx_sb[:, 1, q, :], num_idxs=cfg.HSW,
                            num_idxs_reg=reg, elem_size=F)
                    # pall: [0:256] weighted sum | [256:260] denominator
                    pall = psW.tile([128, 260], F32, tag="pall")
                    for j in range(J):
                        ed = ea_sb[:, j * T_E:(j + 1) * T_E]
                        drow = psS.tile([128, T_E], F32, tag="tmp")
                        nc.tensor.matmul(drow[:], lhsT=ones_sb[EF:EF + 1, :],
                                         rhs=ed[EF:EF + 1, :],
                                         start=True, stop=True,
                                         skip_group_check=True)
                        st_j = selp.tile([128, T_E], F16, tag="st")
                        nc.vector.tensor_tensor(
                            out=st_j[:],
                            in0=iota_col[:].to_broadcast([128, T_E]),
                            in1=drow[:], op=mybir.AluOpType.is_equal)
                        s_j = selp.tile([128, G, 128], F16, tag="s")
                        for g in range(G):
                            nc.vector.tensor_tensor(
                                out=s_j[:, g, :],
                                in0=dstf_sb[:, G * j + g:G * j + g + 1]
                                    .to_broadcast([128, 128]),
                                in1=iota_row[:], op=mybir.AluOpType.is_equal)
                        m_t = mpool.tile([128, 2, T_E], F16, tag="m")
                        for h in range(2):
                            pm = psA.tile([128, T_E], F32, tag="pm")
                            nc.tensor.matmul(
                                pm[:], lhsT=we_sb[:, h * 128:(h + 1) * 128],
                                rhs=ed[:EF, :], start=True, stop=False,
                                skip_group_check=True)
                            nc.tensor.matmul(
                                pm[:],
                                lhsT=xr_sb[:, w, h * 128:(h + 1) * 128],
                                rhs=st_j[:], start=False, stop=True,
                                skip_group_check=True)
                            stT = psA.tile([128, T_E], F16, tag="stT")
                            for g in range(G):
                                nc.tensor.matmul(
                                    stT[:, g * 128:(g + 1) * 128],
                                    lhsT=stage[:, G * j + g,
                                               h * 128:(h + 1) * 128],
                                    rhs=ident16[:], is_transpose=True,
                                    start=True, stop=True,
                                    skip_group_check=True)
                            tsb = mpool.tile([128, T_E], F16, tag="tsb")
                            nc.scalar.activation(
                                tsb[:], stT[:],
                                mybir.ActivationFunctionType.Copy)
                            u = mpool.tile([128, T_E], F16, tag="u")
                            nc.vector.tensor_add(out=u[:], in0=pm[:],
                                                 in1=tsb[:])
                            nc.vector.scalar_tensor_tensor(
                                out=m_t[:, h, :], in0=u[:], scalar=NEG_SLOPE,
                                in1=u[:], op0=mybir.AluOpType.mult,
                                op1=mybir.AluOpType.max)
                        plog = psS.tile([128, 16], F32, tag="tmp")
                        for g in range(G):
                            for h in range(2):
                                nc.tensor.matmul(
                                    plog[:, 4 * g:4 * g + 4],
                                    lhsT=m_t[:, h, g * 128:(g + 1) * 128],
                                    rhs=att_sb[:, h, :],
                                    start=(h == 0), stop=(h == 1),
                                    skip_group_check=True)
                        ex = small.tile([128, 16], F16, tag="ex")
                        nc.scalar.activation(ex[:], plog[:],
                                             mybir.ActivationFunctionType.Exp)
                        v = small.tile([128, G, 260], F16, tag="v")
                        nc.vector.tensor_tensor(
                            out=v[:, :, :256].rearrange(
                                "p g (h c) -> p g h c", h=4),
                            in0=stage[:, G * j:G * (j + 1), :].rearrange(
                                "p g (h c) -> p g h c", h=4),
                            in1=ex.rearrange("p (g h) -> p g h", g=4)
                                .unsqueeze(-1).to_broadcast([128, G, 4, 64]),
                            op=mybir.AluOpType.mult)
                        nc.vector.tensor_copy(
                            out=v[:, :, 256:260],
                            in_=ex.rearrange("p (g h) -> p g h", g=4))
                        for g in range(G):
                            nc.tensor.matmul(
                                pall[:, 0:260], lhsT=s_j[:, g, :],
                                rhs=v[:, g, :],
                                start=(j == 0 and g == 0),
                                stop=(j == J - 1 and g == G - 1),
                                skip_group_check=True)
                    rdf = small.tile([128, 4], F32, tag="rdf")
                    nc.vector.tensor_scalar_add(out=rdf[:],
                                                in0=pall[:, 256:260],
                                                scalar1=EPS)
                    rden = small.tile([128, 4], F32, tag="rden")
                    nc.vector.reciprocal(out=rden[:], in_=rdf[:])
                    hs = small.tile([128, F], F32, tag="hs")
                    nc.vector.tensor_tensor(
                        out=hs.rearrange("p (h c) -> p h c", h=4),
                        in0=pall[:, 0:256].rearrange("p (h c) -> p h c", h=4),
                        in1=rden.unsqueeze(-1).to_broadcast([128, 4, 64]),
                        op=mybir.AluOpType.mult)
                    hb = small.tile([128, F], F32, tag="hb")
                    nc.vector.tensor_add(out=hb[:], in0=hs[:], in1=bias_sb[:])
                    if layer == 0:
                        nc.scalar.activation(h1_sb[:, w, :], hb[:],
                                             mybir.ActivationFunctionType.Relu)
                    else:
                        h_out = small.tile([128, F], F16, tag="hout")
                        nc.scalar.activation(h_out[:], hb[:],
                                             mybir.ActivationFunctionType.Relu)
                        po = psS.tile([128, 1], F32, tag="tmp")
                        for h in range(2):
                            pt = psA.tile([128, 128], F16, tag="stT")
                            nc.tensor.matmul(pt[:],
                                             lhsT=h_out[:, h * 128:(h + 1) * 128],
                                             rhs=ident16[:], is_transpose=True,
                                             start=True, stop=True,
                                             skip_group_check=True)
                            h2T = small.tile([128, 128], F16, tag="h2T")
                            nc.vector.tensor_copy(out=h2T[:], in_=pt[:])
                            nc.tensor.matmul(po[:], lhsT=h2T[:],
                                             rhs=wout_sb[:, h, :],
                                             start=(h == 0), stop=(h == 1),
                                             skip_group_check=True)
                        o_sb = small.tile([128, 1], F32, tag="osb")
                        nc.vector.tensor_scalar(
                            out=o_sb[:], in0=po[:], scalar1=bout_sb[:, :1],
                            scalar2=None, op0=mybir.AluOpType.add)
                        nc.sync.dma_start(out=out_own[w * W:(w + 1) * W, :],
                                          in_=o_sb[:])

            # ---------------- layer-1 projections ----------------
            def phase_p4():
                for c in range(NW):
                    h1T = io.tile([128, 2, 128], F16, tag="h1T")
                    for h in range(2):
                        pt = psA.tile([128, 128], F16, tag="stT")
                        nc.tensor.matmul(pt[:],
                                         lhsT=h1_sb[:, c, h * 128:(h + 1) * 128],
                                         rhs=ident16[:], is_transpose=True,
                                         start=True, stop=True,
                                         skip_group_check=True)
                        nc.vector.tensor_copy(out=h1T[:, h, :], in_=pt[:])
                    plr = psA.tile([128, 2, F], F32, tag="pm")
                    for h in range(2):
                        nc.tensor.matmul(plr[:, 0, :], lhsT=h1T[:, h, :],
                                         rhs=wl1_sb[:, h, :],
                                         start=(h == 0), stop=(h == 1),
                                         skip_group_check=True)
                    for h in range(2):
                        nc.tensor.matmul(plr[:, 1, :], lhsT=h1T[:, h, :],
                                         rhs=wr1_sb[:, h, :],
                                         start=(h == 0), stop=(h == 1),
                                         skip_group_check=True)
                    ol = io.tile([128, F], F16, tag="oxl")
                    nc.vector.tensor_add(out=ol[:], in0=plr[:, 0, :],
                                         in1=bsb["bl1"][:])
                    nc.vector.tensor_add(out=xr_sb[:, c, :], in0=plr[:, 1, :],
                                         in1=bsb["br1"][:])
                    nc.sync.dma_start(out=xl1_own[c * 128:(c + 1) * 128, :],
                                      in_=ol[:])

            def phase_ag(xl_own, xl_full):
                tc.strict_bb_all_engine_barrier()
                nc.gpsimd.collective_compute(
                    "AllGather", mybir.AluOpType.bypass, replica_groups=groups,
                    ins=[xl_own[:]], outs=[xl_full[:]])
                tc.strict_bb_all_engine_barrier()

            for _rep in range(cfg.reps):
                if "p1" in cfg.phases:
                    phase_p1()
                if "ag0" in cfg.phases:
                    phase_ag(xl0_own, xl0_full)
                if "e0" in cfg.phases:
                    edge_pass(0, xl0_full, we0_sb, batt["att0"], bsb["bias0"])
                tc.strict_bb_all_engine_barrier()
                if "p4" in cfg.phases:
                    phase_p4()
                if "ag1" in cfg.phases:
                    phase_ag(xl1_own, xl1_full)
                if "e1" in cfg.phases:
                    edge_pass(1, xl1_full, we1_sb, batt["att1"], bsb["bias1"])
                if cfg.reps > 1:
                    tc.strict_bb_all_engine_barrier()
    return P


_CACHE = {}


def _get_compiled(cfg):
    key = (cfg.N, cfg.E, cfg.IN_F, cfg.NC, cfg.J, cfg.reps, cfg.phases)
    if key not in _CACHE:
        nc = bacc.Bacc("TRN2", target_bir_lowering=False, debug=False,
                       num_devices=cfg.NC, dynamic_dma_scratch_size=49152)
        build_program(cfg, nc)
        nc.compile()
        _CACHE[key] = nc
    return _CACHE[key]


def make_in_maps(cfg, inputs, cores_pre):
    """Per-core input dicts."""
    x = np.asarray(inputs["x"], np.float32)
    H, C, F = cfg.H, cfg.C, cfg.F
    att_blk = {}
    for li in (0, 1):
        att = np.asarray(inputs[f"att{li}"], np.float32)   # [H, C]
        A = np.zeros((2 * 128, 4), np.float32)
        for h in range(H):
            A[h * C:(h + 1) * C, h] = att[h]
        att_blk[li] = np.ascontiguousarray(
            A.reshape(2, 128, 4).transpose(1, 0, 2)).astype(np.float16)
    iota_r16 = np.tile(np.arange(128, dtype=np.float16)[None, :], (128, 1))
    iota_c = np.arange(128, dtype=np.float32).reshape(128, 1)
    ones16 = np.ones((33, 128), np.float16)
    rep = lambda v: np.tile(np.asarray(v, np.float32)[None, :], (128, 1))
    f16 = lambda v: np.asarray(v, np.float32).astype(np.float16)
    common = dict(
        wl0=f16(inputs["W_l0"]), wr0=f16(inputs["W_r0"]),
        we0=f16(inputs["W_e0"]), wl1=f16(inputs["W_l1"]),
        wr1=f16(inputs["W_r1"]), we1=f16(inputs["W_e1"]),
        wout=f16(inputs["W_out"]).reshape(2, 128, 1).transpose(1, 0, 2).copy(),
        att0=att_blk[0], att1=att_blk[1],
        bl0=rep(inputs["b_l0"]), br0=rep(inputs["b_r0"]),
        bias0=rep(inputs["bias0"]), bl1=rep(inputs["b_l1"]),
        br1=rep(inputs["b_r1"]), bias1=rep(inputs["bias1"]),
        bout=np.tile(np.asarray(inputs["b_out"], np.float32).reshape(1, 1),
                     (128, 1)),
        iota_r16=iota_r16, iota_c=iota_c, ones16=ones16,
        ident_in=np.eye(128, dtype=np.float16),
    )
    in_maps = []
    for c in range(cfg.NC):
        pre = cores_pre[c]
        xs = np.zeros((cfg.NVP, cfg.IN_F), np.float32)
        xs[:cfg.NV] = x[c * cfg.NV:(c + 1) * cfg.NV]
        m = dict(common)
        m.update(x_T=np.ascontiguousarray(xs.T).astype(np.float16),
                 idx2=pre["idx2"], klow=pre["klow"], dstf=pre["dstf"],
                 eat=pre["eat"])
        in_maps.append(m)
    return in_maps


def _run(cfg, inputs):
    cores_pre = preprocess(cfg, inputs["edge_index"], inputs["edge_attr"])
    in_maps = make_in_maps(cfg, inputs, cores_pre)
    nc = _get_compiled(cfg)
    res = run_bass_kernel_spmd(nc, in_maps, core_ids=list(range(cfg.NC)))
    outs = []
    for c in range(cfg.NC):
        outs.append(res.results[c]["out_own"][:cfg.NV])
    return np.concatenate(outs, 0).astype(np.float32)


def kernel(**inputs):
    cfg = Cfg(N=50000, E=800000, IN_F=512, NC=8, J=5)
    # bump J if some window overflows (keeps NEFF cache stable otherwise)
    dst = np.asarray(inputs["edge_index"][1], np.int64)
    loc = dst % cfg.NV
    wid = (dst // cfg.NV) * cfg.NW + loc // cfg.W
    need = np.bincount(wid, minlength=cfg.NC * cfg.NW).max()
    while need > cfg.SW - cfg.NCH:
        cfg = Cfg(N=50000, E=800000, IN_F=512, NC=8, J=cfg.J + 1)
    return _run(cfg, inputs)


# revision 43
# speedup vs baseline: 15.0313x; 1.1437x over previous
"""Trainium2 Bass kernel for 2-layer GATv2 + output projection (SPMD, 8 cores).

v2 strategy: nodes partitioned across cores; per 128-dst-node window, edges
are packed into SW=J*512 slots (sorted by gathered-row id so the low/high
int16 gather split is two window-level dma_gather calls). fp16 data plane
(same mantissa as TF32/f32r), f32 PSUM accumulation. Softmax denominator is
factored out of the weighted scatter (out = (sum ex*xl[src]) / denom), so
each window is a single pass: per 512-slot tile, edge-feature projection +
xr[dst] broadcast (selector matmul) + transposed gathered xl accumulate into
PSUM; fused leaky-relu; per-head logits via PE; exp on ACT; one selector
matmul per 128-slot group scatters both the weighted values and the
denominator (values carry ex in 4 extra columns). Halo exchange = AllGather
of fp16 xl shards. xr and the inter-layer h1 stay SBUF-resident.
"""
import numpy as np

import concourse.bass as bass
import concourse.bacc as bacc
import concourse.mybir as mybir
import concourse.tile as tile
from concourse.bass_utils import run_bass_kernel_spmd
from concourse.masks import make_identity

F32 = mybir.dt.float32
F16 = mybir.dt.float16
I16 = mybir.dt.int16
I32 = mybir.dt.int32

NEG_SLOPE = 0.2
EPS = 1e-30


class Cfg:
    def __init__(self, N, E, IN_F, NC, J, reps=1):
        self.N, self.E, self.IN_F, self.NC, self.J = N, E, IN_F, NC, J
        self.reps = reps
        self.F = 256
        self.H, self.C = 4, 64
        self.W = 128                      # dst nodes per window
        assert N % NC == 0
        self.NV = N // NC                 # nodes per core
        self.NW = (self.NV + self.W - 1) // self.W
        self.NVP = self.NW * self.W       # padded nodes per core
        self.T_E = 512                    # edge slots per tile
        self.G = 4                        # 128-edge groups per tile
        self.SW = self.J * self.T_E       # edge slots per window
        self.SWC = self.SW // 128         # 128-slot chunks per window
        import os
        self.NCH = int(os.environ.get("K_GCH", "4"))  # gather chunks/window
        assert self.SWC % self.NCH == 0
        self.HSW = self.SW // self.NCH    # slots per gather chunk
        assert self.HSW % 128 == 0
        self.NFULL = self.NC * self.NVP   # rows in gathered xl table
        assert self.NFULL % 2 == 0
        self.HALF = self.NFULL // 2
        assert self.HALF <= 32768, "int16 gather index range"
        self.KCH = self.IN_F // 128       # K-chunks for layer-0 projection
        import os
        _ph = os.environ.get("K_PHASES", "p1,ag0,e0,p4,ag1,e1")
        self.phases = tuple(x for x in _ph.split(",") if x)


def preprocess(cfg, edge_index, edge_attr):
    """Sort edges by dst window, pack into window slots, build per-core arrays."""
    src = np.asarray(edge_index[0], np.int64)
    dst = np.asarray(edge_index[1], np.int64)
    ea = np.asarray(edge_attr, np.float32)
    NV, W, NW, SW, SWC = cfg.NV, cfg.W, cfg.NW, cfg.SW, cfg.SWC
    EF = ea.shape[1]
    # padded-global row of each node's xl entry
    core_of = src // NV
    grow = core_of * cfg.NVP + (src - core_of * NV)

    order = np.argsort(dst, kind="stable")
    wrap = lambda a: np.tile(a.reshape(len(a) // 16, 16).T, (8, 1))
    cores = []
    for c in range(cfg.NC):
        lo = np.searchsorted(dst, c * NV, side="left", sorter=order)
        hi = np.searchsorted(dst, (c + 1) * NV, side="left", sorter=order)
        eidx_c = order[lo:hi]
        dloc = dst[eidx_c] - c * NV
        win = dloc // W

        HSW, NCH = cfg.HSW, cfg.NCH
        idx2 = np.zeros((NW, 128, 2, NCH, HSW // 16), np.int16)
        klow = np.zeros((1, NCH * NW), np.int32)
        dstf = np.zeros((NW, 128, SWC), np.float16)
        eat = np.zeros((NW, EF + 1, SW), np.float16)

        for w in range(NW):
            e_w = eidx_c[win == w]
            g_w = grow[e_w]
            o = np.argsort(g_w, kind="stable")
            e_w, g_w = e_w[o], g_w[o]
            n_e = len(e_w)
            cap = HSW - 1
            assert n_e <= NCH * cap, (
                f"window overflow: {n_e} > {NCH * cap}; raise J")
            drow = np.full(SW, -1.0, np.float16)
            eaT = np.zeros((EF, SW), np.float16)
            dl = (dst[e_w] - c * NV - w * W).astype(np.float16)
            for q in range(NCH):
                e_q = e_w[q * cap:(q + 1) * cap]
                g_q = g_w[q * cap:(q + 1) * cap]
                d_q = dl[q * cap:(q + 1) * cap]
                n_q = len(e_q)
                n_lo = int((g_q < cfg.HALF).sum())
                kl = n_lo + 1             # + leading dummy slot
                idx_lo = np.full(HSW, -1, np.int16)
                idx_lo[0] = 0
                idx_lo[1:kl] = g_q[:n_lo].astype(np.int16)
                idx_hi = np.zeros(HSW, np.int16)
                idx_hi[kl:kl + (n_q - n_lo)] = (g_q[n_lo:] - cfg.HALF
                                                ).astype(np.int16)
                idx2[w, :, 0, q, :] = wrap(idx_hi)
                idx2[w, :, 1, q, :] = wrap(idx_lo)
                klow[0, NCH * w + q] = kl
                s0 = q * HSW
                drow[s0 + 1:s0 + kl] = d_q[:n_lo]
                drow[s0 + kl:s0 + kl + (n_q - n_lo)] = d_q[n_lo:]
                eaT[:, s0 + 1:s0 + kl] = ea[e_q[:n_lo]].T
                eaT[:, s0 + kl:s0 + kl + (n_q - n_lo)] = ea[e_q[n_lo:]].T
            dstf[w] = drow.reshape(SWC, 128).T
            eat[w, EF, :] = drow
            eat[w, :EF, :] = eaT
        cores.append(dict(idx2=idx2, klow=klow, dstf=dstf, eat=eat))
    return cores


def build_program(cfg, nc):
    """Emit the full SPMD program into nc (a Bacc) under TileContext."""
    F, G, T_E, J, NW, SW, SWC, NVP, W = (cfg.F, cfg.G, cfg.T_E, cfg.J, cfg.NW,
                                         cfg.SW, cfg.SWC, cfg.NVP, cfg.W)
    EF = 32
    P = {}
    def inp(name, shape, dt):
        P[name] = nc.dram_tensor(name, shape, dt, kind="ExternalInput")
        return P[name]

    x_T = inp("x_T", [cfg.IN_F, NVP], F16)
    idx2 = inp("idx2", [NW, 128, 2, cfg.NCH, cfg.HSW // 16], I16)
    klow = inp("klow", [1, cfg.NCH * NW], I32)
    dstf = inp("dstf", [NW, 128, SWC], F16)
    eat = inp("eat", [NW, EF + 1, SW], F16)
    wl0 = inp("wl0", [cfg.IN_F, F], F16)
    wr0 = inp("wr0", [cfg.IN_F, F], F16)
    we0 = inp("we0", [EF, F], F16)
    wl1 = inp("wl1", [F, F], F16)
    wr1 = inp("wr1", [F, F], F16)
    we1 = inp("we1", [EF, F], F16)
    wout = inp("wout", [128, 2, 1], F16)
    att0 = inp("att0", [128, 2, 4], F16)
    att1 = inp("att1", [128, 2, 4], F16)
    bl0 = inp("bl0", [128, F], F32)
    br0 = inp("br0", [128, F], F32)
    bias0 = inp("bias0", [128, F], F32)
    bl1 = inp("bl1", [128, F], F32)
    br1 = inp("br1", [128, F], F32)
    bias1 = inp("bias1", [128, F], F32)
    bout = inp("bout", [128, 1], F32)
    iota_r16 = inp("iota_r16", [128, 128], F16)
    iota_c = inp("iota_c", [128, 1], F32)
    ones16 = inp("ones16", [33, 128], F16)
    ident_in = inp("ident_in", [128, 128], F16)

    out_own = nc.dram_tensor("out_own", [NVP, 1], F32, kind="ExternalOutput")

    # ---- internal DRAM
    xl0_own = nc.dram_tensor("xl0_own", [NVP, F], F16)
    xl1_own = nc.dram_tensor("xl1_own", [NVP, F], F16)
    akw = dict(addr_space="Shared") if cfg.NC > 4 else {}
    xl0_full = nc.dram_tensor("xl0_full", [cfg.NFULL, F], F16, **akw)
    xl1_full = nc.dram_tensor("xl1_full", [cfg.NFULL, F], F16, **akw)
    groups = [list(range(cfg.NC))]

    with tile.TileContext(nc) as tc:
        with (
            tc.tile_pool(name="const", bufs=1) as constp,
            tc.tile_pool(name="wpool", bufs=1) as wpool,
            tc.tile_pool(name="resid", bufs=1) as resid,
            tc.tile_pool(name="io", bufs=3) as io,
            tc.tile_pool(name="stg", bufs=2) as stg,
            tc.tile_pool(name="sel", bufs=3) as selp,
            tc.tile_pool(name="mpool", bufs=2) as mpool,
            tc.tile_pool(name="small", bufs=4) as small,
            tc.tile_pool(name="psA", bufs=2, space="PSUM") as psA,
            tc.tile_pool(name="psW", bufs=2, space="PSUM") as psW,
            tc.tile_pool(name="psS", bufs=2, space="PSUM") as psS,
        ):
            ident16 = constp.tile([128, 128], F16)
            nc.sync.dma_start(out=ident16[:], in_=ident_in[:])
            iota_row = constp.tile([128, 128], F16)
            nc.sync.dma_start(out=iota_row[:], in_=iota_r16[:])
            iota_col = constp.tile([128, 1], F32)
            nc.sync.dma_start(out=iota_col[:], in_=iota_c[:])
            ones_sb = constp.tile([33, 128], F16)
            nc.sync.dma_start(out=ones_sb[:], in_=ones16[:])
            klow_sb = constp.tile([1, cfg.NCH * NW], I32)
            nc.sync.dma_start(out=klow_sb[:], in_=klow[:])
            batt = {}
            for nm, t in (("att0", att0), ("att1", att1)):
                bt = constp.tile([128, 2, 4], F16, tag=nm)
                nc.sync.dma_start(out=bt[:], in_=t[:])
                batt[nm] = bt
            bout_sb = constp.tile([128, 1], F32)
            nc.sync.dma_start(out=bout_sb[:], in_=bout[:])
            bsb = {}
            for nm, t in (("bl0", bl0), ("br0", br0), ("bias0", bias0),
                          ("bl1", bl1), ("br1", br1), ("bias1", bias1)):
                bt = constp.tile([128, F], F32, tag=nm)
                nc.sync.dma_start(out=bt[:], in_=t[:])
                bsb[nm] = bt

            def load_w(t, kdim, tag):
                n = kdim // 128
                w = wpool.tile([128, n, F], F16, tag=tag)
                nc.sync.dma_start(
                    out=w[:], in_=t.rearrange("(k p) f -> p k f", p=128))
                return w
            wl0_sb = load_w(wl0, cfg.IN_F, "wl0")
            wr0_sb = load_w(wr0, cfg.IN_F, "wr0")
            wl1_sb = load_w(wl1, F, "wl1")
            wr1_sb = load_w(wr1, F, "wr1")
            we0_sb = wpool.tile([EF, F], F16, tag="we0")
            nc.sync.dma_start(out=we0_sb[:], in_=we0[:])
            we1_sb = wpool.tile([EF, F], F16, tag="we1")
            nc.sync.dma_start(out=we1_sb[:], in_=we1[:])
            wout_sb = wpool.tile([128, 2, 1], F16, tag="wout")
            nc.sync.dma_start(out=wout_sb[:], in_=wout[:])

            # SBUF-resident xr (shared by both layers) and inter-layer h1
            xr_sb = resid.tile([128, NW, F], F16, tag="xr")
            h1_sb = resid.tile([128, NW, F], F16, tag="h1")

            # ---------------- layer-0 projections ----------------
            def phase_p1():
                for c in range(NW):
                    xk = io.tile([128, cfg.KCH, 128], F16, tag="xk")
                    nc.sync.dma_start(
                        out=xk[:],
                        in_=x_T.rearrange("(k p) n -> p k n", p=128)
                            [:, :, c * 128:(c + 1) * 128])
                    plr = psA.tile([128, 2, F], F32, tag="pm", bufs=4)
                    for k in range(cfg.KCH):
                        nc.tensor.matmul(plr[:, 0, :], lhsT=xk[:, k, :],
                                         rhs=wl0_sb[:, k, :],
                                         start=(k == 0), stop=(k == cfg.KCH - 1),
                                         skip_group_check=True)
                    for k in range(cfg.KCH):
                        nc.tensor.matmul(plr[:, 1, :], lhsT=xk[:, k, :],
                                         rhs=wr0_sb[:, k, :],
                                         start=(k == 0), stop=(k == cfg.KCH - 1),
                                         skip_group_check=True)
                    ol = io.tile([128, F], F16, tag="oxl")
                    nc.vector.tensor_add(out=ol[:], in0=plr[:, 0, :],
                                         in1=bsb["bl0"][:])
                    nc.vector.tensor_add(out=xr_sb[:, c, :], in0=plr[:, 1, :],
                                         in1=bsb["br0"][:])
                    nc.sync.dma_start(out=xl0_own[c * 128:(c + 1) * 128, :],
                                      in_=ol[:])

            # ---------------- edge pass (shared for both layers) ------------
            def edge_pass(layer, xl_full, we_sb, att_sb, bias_sb, cc=None):
                NCH = cfg.NCH
                HC = SWC // NCH
                for w in range(NW):
                    idx_sb = io.tile([128, 2, NCH, cfg.HSW // 16], I16,
                                     tag="idx")
                    nc.sync.dma_start(out=idx_sb[:], in_=idx2[w])
                    ea_sb = io.tile([EF + 1, SW], F16, tag="ea")
                    nc.sync.dma_start(out=ea_sb[:], in_=eat[w])
                    dstf_sb = io.tile([128, SWC], F16, tag="dstf")
                    nc.sync.dma_start(out=dstf_sb[:], in_=dstf[w])
                    stage = stg.tile([128, SWC, F], F16, tag="stage")
                    for q in range(NCH):
                        reg = nc.gpsimd.alloc_register()
                        nc.gpsimd.load(
                            reg, klow_sb[0:1, NCH * w + q:NCH * w + q + 1])
                        g1 = nc.gpsimd.dma_gather(
                            out_ap=stage[:, q * HC:(q + 1) * HC, :],
                            in_ap=xl_full[cfg.HALF:, :],
                            idxs_ap=idx_sb[:, 0, q, :], num_idxs=cfg.HSW,
                            num_idxs_reg=cfg.HSW, elem_size=F)
                        g2 = nc.gpsimd.dma_gather(
                            out_ap=stage[:, q * HC:(q + 1) * HC, :],
                            in_ap=xl_full[:cfg.HALF, :],
                            idxs_ap=idx_sb[:, 1, q, :], num_idxs=cfg.HSW,
                            num_idxs_reg=reg, elem_size=F)
                        if cc is not None:
                            for g_i in (g1, g2):
                                bass._add_dep_helper(
                                    g_i.ins, cc.ins, sync=True,
                                    reason="gather reads AllGather output")
                    # pall: [0:256] weighted sum | [256:260] denominator
                    pall = psW.tile([128, 260], F32, tag="pall")
                    for j in range(J):
                        ed = ea_sb[:, j * T_E:(j + 1) * T_E]
                        drow = psS.tile([128, T_E], F32, tag="tmp")
                        nc.tensor.matmul(drow[:], lhsT=ones_sb[EF:EF + 1, :],
                                         rhs=ed[EF:EF + 1, :],
                                         start=True, stop=True,
                                         skip_group_check=True)
                        st_j = selp.tile([128, T_E], F16, tag="st")
                        nc.vector.tensor_tensor(
                            out=st_j[:],
                            in0=iota_col[:].to_broadcast([128, T_E]),
                            in1=drow[:], op=mybir.AluOpType.is_equal)
                        s_j = selp.tile([128, G, 128], F16, tag="s")
                        for g in range(G):
                            nc.vector.tensor_tensor(
                                out=s_j[:, g, :],
                                in0=dstf_sb[:, G * j + g:G * j + g + 1]
                                    .to_broadcast([128, 128]),
                                in1=iota_row[:], op=mybir.AluOpType.is_equal)
                        m_t = mpool.tile([128, 2, T_E], F16, tag="m")
                        for h in range(2):
                            pm = psA.tile([128, T_E], F32, tag="pm", bufs=4)
                            nc.tensor.matmul(
                                pm[:], lhsT=we_sb[:, h * 128:(h + 1) * 128],
                                rhs=ed[:EF, :], start=True, stop=False,
                                skip_group_check=True)
                            nc.tensor.matmul(
                                pm[:],
                                lhsT=xr_sb[:, w, h * 128:(h + 1) * 128],
                                rhs=st_j[:], start=False, stop=False,
                                skip_group_check=True)
                            # transpose-accumulate gathered xl[src] via
                            # identity matmul (stage^T @ I), f32 accumulate
                            for g in range(G):
                                nc.tensor.matmul(
                                    pm[:, g * 128:(g + 1) * 128],
                                    lhsT=stage[:, G * j + g,
                                               h * 128:(h + 1) * 128],
                                    rhs=ident16[:],
                                    start=False, stop=(g == G - 1),
                                    skip_group_check=True)
                            rp = mpool.tile([128, T_E], F16, tag="rp")
                            nc.scalar.activation(
                                rp[:], pm[:], mybir.ActivationFunctionType.Relu,
                                scale=1.0 - NEG_SLOPE)
                            nc.vector.scalar_tensor_tensor(
                                out=m_t[:, h, :], in0=pm[:], scalar=NEG_SLOPE,
                                in1=rp[:], op0=mybir.AluOpType.mult,
                                op1=mybir.AluOpType.add)
                        plog = psS.tile([128, 16], F32, tag="tmp")
                        for g in range(G):
                            for h in range(2):
                                nc.tensor.matmul(
                                    plog[:, 4 * g:4 * g + 4],
                                    lhsT=m_t[:, h, g * 128:(g + 1) * 128],
                                    rhs=att_sb[:, h, :],
                                    start=(h == 0), stop=(h == 1),
                                    skip_group_check=True)
                        ex = small.tile([128, 16], F16, tag="ex")
                        nc.scalar.activation(ex[:], plog[:],
                                             mybir.ActivationFunctionType.Exp)
                        v = small.tile([128, G, 260], F16, tag="v")
                        nc.vector.tensor_tensor(
                            out=v[:, :, :256].rearrange(
                                "p g (h c) -> p g h c", h=4),
                            in0=stage[:, G * j:G * (j + 1), :].rearrange(
                                "p g (h c) -> p g h c", h=4),
                            in1=ex.rearrange("p (g h) -> p g h", g=4)
                                .unsqueeze(-1).to_broadcast([128, G, 4, 64]),
                            op=mybir.AluOpType.mult)
                        nc.vector.tensor_copy(
                            out=v[:, :, 256:260],
                            in_=ex.rearrange("p (g h) -> p g h", g=4))
                        for g in range(G):
                            nc.tensor.matmul(
                                pall[:, 0:260], lhsT=s_j[:, g, :],
                                rhs=v[:, g, :],
                                start=(j == 0 and g == 0),
                                stop=(j == J - 1 and g == G - 1),
                                skip_group_check=True)
                    rdf = small.tile([128, 4], F32, tag="rdf")
                    nc.vector.tensor_scalar_add(out=rdf[:],
                                                in0=pall[:, 256:260],
                                                scalar1=EPS)
                    rden = small.tile([128, 4], F32, tag="rden")
                    nc.vector.reciprocal(out=rden[:], in_=rdf[:])
                    hs = small.tile([128, F], F32, tag="hs")
                    nc.vector.tensor_tensor(
                        out=hs.rearrange("p (h c) -> p h c", h=4),
                        in0=pall[:, 0:256].rearrange("p (h c) -> p h c", h=4),
                        in1=rden.unsqueeze(-1).to_broadcast([128, 4, 64]),
                        op=mybir.AluOpType.mult)
                    hb = small.tile([128, F], F32, tag="hb")
                    nc.vector.tensor_add(out=hb[:], in0=hs[:], in1=bias_sb[:])
                    if layer == 0:
                        nc.scalar.activation(h1_sb[:, w, :], hb[:],
                                             mybir.ActivationFunctionType.Relu)
                    else:
                        h_out = small.tile([128, F], F16, tag="hout")
                        nc.scalar.activation(h_out[:], hb[:],
                                             mybir.ActivationFunctionType.Relu)
                        po = psS.tile([128, 1], F32, tag="tmp")
                        for h in range(2):
                            pt = psS.tile([128, 128], F16, tag="tmp")
                            nc.tensor.matmul(pt[:],
                                             lhsT=h_out[:, h * 128:(h + 1) * 128],
                                             rhs=ident16[:], is_transpose=True,
                                             start=True, stop=True,
                                             skip_group_check=True)
                            h2T = small.tile([128, 128], F16, tag="h2T")
                            nc.vector.tensor_copy(out=h2T[:], in_=pt[:])
                            nc.tensor.matmul(po[:], lhsT=h2T[:],
                                             rhs=wout_sb[:, h, :],
                                             start=(h == 0), stop=(h == 1),
                                             skip_group_check=True)
                        o_sb = small.tile([128, 1], F32, tag="osb")
                        nc.vector.tensor_scalar(
                            out=o_sb[:], in0=po[:], scalar1=bout_sb[:, :1],
                            scalar2=None, op0=mybir.AluOpType.add)
                        nc.sync.dma_start(out=out_own[w * W:(w + 1) * W, :],
                                          in_=o_sb[:])

            # ---------------- layer-1 projections ----------------
            def phase_p4():
                for c in range(NW):
                    h1T = io.tile([128, 2, 128], F16, tag="h1T")
                    for h in range(2):
                        pt = psS.tile([128, 128], F16, tag="tmp")
                        nc.tensor.matmul(pt[:],
                                         lhsT=h1_sb[:, c, h * 128:(h + 1) * 128],
                                         rhs=ident16[:], is_transpose=True,
                                         start=True, stop=True,
                                         skip_group_check=True)
                        nc.vector.tensor_copy(out=h1T[:, h, :], in_=pt[:])
                    plr = psA.tile([128, 2, F], F32, tag="pm", bufs=4)
                    for h in range(2):
                        nc.tensor.matmul(plr[:, 0, :], lhsT=h1T[:, h, :],
                                         rhs=wl1_sb[:, h, :],
                                         start=(h == 0), stop=(h == 1),
                                         skip_group_check=True)
                    for h in range(2):
                        nc.tensor.matmul(plr[:, 1, :], lhsT=h1T[:, h, :],
                                         rhs=wr1_sb[:, h, :],
                                         start=(h == 0), stop=(h == 1),
                                         skip_group_check=True)
                    ol = io.tile([128, F], F16, tag="oxl")
                    nc.vector.tensor_add(out=ol[:], in0=plr[:, 0, :],
                                         in1=bsb["bl1"][:])
                    nc.vector.tensor_add(out=xr_sb[:, c, :], in0=plr[:, 1, :],
                                         in1=bsb["br1"][:])
                    nc.sync.dma_start(out=xl1_own[c * 128:(c + 1) * 128, :],
                                      in_=ol[:])

            def phase_ag(xl_own, xl_full):
                # Barrier ensures the xl shard DMA writes landed before the
                # collective reads them. No post-AG barrier: the only
                # consumers of xl_full are the dma_gathers, which follow the
                # collective in GPSIMD program order — so window preambles
                # (loads, selector builds) overlap the collective.
                tc.strict_bb_all_engine_barrier()
                return nc.gpsimd.collective_compute(
                    "AllGather", mybir.AluOpType.bypass, replica_groups=groups,
                    ins=[xl_own[:]], outs=[xl_full[:]])

            for _rep in range(cfg.reps):
                cc0 = cc1 = None
                if "p1" in cfg.phases:
                    phase_p1()
                if "ag0" in cfg.phases:
                    cc0 = phase_ag(xl0_own, xl0_full)
                if "e0" in cfg.phases:
                    edge_pass(0, xl0_full, we0_sb, batt["att0"], bsb["bias0"],
                              cc=cc0)
                if "p4" in cfg.phases:
                    phase_p4()
                if "ag1" in cfg.phases:
                    cc1 = phase_ag(xl1_own, xl1_full)
                if "e1" in cfg.phases:
                    edge_pass(1, xl1_full, we1_sb, batt["att1"], bsb["bias1"],
                              cc=cc1)
                # Rep-end barrier: guarantees the collectives' reads and all
                # in-flight gather DMAs completed before the next rep (or
                # program end) rewrites their sources.
                tc.strict_bb_all_engine_barrier()
    return P


_CACHE = {}


def _get_compiled(cfg):
    key = (cfg.N, cfg.E, cfg.IN_F, cfg.NC, cfg.J, cfg.reps, cfg.phases)
    if key not in _CACHE:
        nc = bacc.Bacc("TRN2", target_bir_lowering=False, debug=False,
                       num_devices=cfg.NC, dynamic_dma_scratch_size=49152)
        build_program(cfg, nc)
        nc.compile()
        _CACHE[key] = nc
    return _CACHE[key]


def make_in_maps(cfg, inputs, cores_pre):
    """Per-core input dicts."""
    x = np.asarray(inputs["x"], np.float32)
    H, C, F = cfg.H, cfg.C, cfg.F
    att_blk = {}
    for li in (0, 1):
        att = np.asarray(inputs[f"att{li}"], np.float32)   # [H, C]
        A = np.zeros((2 * 128, 4), np.float32)
        for h in range(H):
            A[h * C:(h + 1) * C, h] = att[h]
        att_blk[li] = np.ascontiguousarray(
            A.reshape(2, 128, 4).transpose(1, 0, 2)).astype(np.float16)
    iota_r16 = np.tile(np.arange(128, dtype=np.float16)[None, :], (128, 1))
    iota_c = np.arange(128, dtype=np.float32).reshape(128, 1)
    ones16 = np.ones((33, 128), np.float16)
    rep = lambda v: np.tile(np.asarray(v, np.float32)[None, :], (128, 1))
    f16 = lambda v: np.asarray(v, np.float32).astype(np.float16)
    common = dict(
        wl0=f16(inputs["W_l0"]), wr0=f16(inputs["W_r0"]),
        we0=f16(inputs["W_e0"]), wl1=f16(inputs["W_l1"]),
        wr1=f16(inputs["W_r1"]), we1=f16(inputs["W_e1"]),
        wout=f16(inputs["W_out"]).reshape(2, 128, 1).transpose(1, 0, 2).copy(),
        att0=att_blk[0], att1=att_blk[1],
        bl0=rep(inputs["b_l0"]), br0=rep(inputs["b_r0"]),
        bias0=rep(inputs["bias0"]), bl1=rep(inputs["b_l1"]),
        br1=rep(inputs["b_r1"]), bias1=rep(inputs["bias1"]),
        bout=np.tile(np.asarray(inputs["b_out"], np.float32).reshape(1, 1),
                     (128, 1)),
        iota_r16=iota_r16, iota_c=iota_c, ones16=ones16,
        ident_in=np.eye(128, dtype=np.float16),
    )
    in_maps = []
    for c in range(cfg.NC):
        pre = cores_pre[c]
        xs = np.zeros((cfg.NVP, cfg.IN_F), np.float32)
        xs[:cfg.NV] = x[c * cfg.NV:(c + 1) * cfg.NV]
        m = dict(common)
        m.update(x_T=np.ascontiguousarray(xs.T).astype(np.float16),
                 idx2=pre["idx2"], klow=pre["klow"], dstf=pre["dstf"],
                 eat=pre["eat"])
        in_maps.append(m)
    return in_maps


def _run(cfg, inputs):
    cores_pre = preprocess(cfg, inputs["edge_index"], inputs["edge_attr"])
    in_maps = make_in_maps(cfg, inputs, cores_pre)
    nc = _get_compiled(cfg)
    res = run_bass_kernel_spmd(nc, in_maps, core_ids=list(range(cfg.NC)))
    outs = []
    for c in range(cfg.NC):
        outs.append(res.results[c]["out_own"][:cfg.NV])
    return np.concatenate(outs, 0).astype(np.float32)


def kernel(**inputs):
    cfg = Cfg(N=50000, E=800000, IN_F=512, NC=8, J=5)
    # bump J if some window overflows (keeps NEFF cache stable otherwise)
    dst = np.asarray(inputs["edge_index"][1], np.int64)
    loc = dst % cfg.NV
    wid = (dst // cfg.NV) * cfg.NW + loc // cfg.W
    need = np.bincount(wid, minlength=cfg.NC * cfg.NW).max()
    while need > cfg.SW - cfg.NCH:
        cfg = Cfg(N=50000, E=800000, IN_F=512, NC=8, J=cfg.J + 1)
    return _run(cfg, inputs)
